# revision 42
# baseline (speedup 1.0000x reference)
"""HalfKP input layer (dual GEMV + bias + relu) on 8 Trainium2 NeuronCores.

out[512] = concat(relu(W_my @ x[:41024] + b_my), relu(W_opp @ x[41024:] + b_opp))

Sharding: 512 output rows split 64 rows/core (cores 0-3: W_my, 4-7: W_opp).
W is quantized to q = round(W * 127 * sqrt(K)) (rel err ~5e-3, gate is 2e-2)
and shipped partly as int8, partly as bf16(q) — same values, one uniform
1/sw dequant at the end.

Per core the 64 rows split across two pipelines that run in parallel
(measured: ACT cast ~0.8-1.0 G elem/s/partition, DVE cast 1.6, DVE TTR 0.83,
PE ~1.13 GHz under the activity throttle; HWDGE DMA ~250 GB/s/ring):

* PE path (rows 0..RP-1): x-stationary GEMV.  K cut into 321 blocks of 128
  (tail zero-padded).  Per chunk of 8 blocks the PE loads 8 x-blocks (bf16)
  stationary and streams the chunk's W as the moving operand [128, 8*RP],
  accumulating psum[i, n] over 40 chunks.  Chunks 16..39 ship as bf16
  directly (no cast, DMA-paced); chunks 0..15 ship int8 and are cast to
  bf16 by the Scalar engine in 4 group-casts, overlapping the bf16 DMA.
  The bf16 chunks are processed FIRST so the PE starts as soon as the
  first bf16 group lands.  Diagonal extraction: one aligned psum->SBUF
  copy + 8 selector matmuls (lhsT = e_j), then a fused scale+bias DVE op
  and relu.

* DVE path (rows RP..63): TENSOR_TENSOR_REDUCE directly on int8 W x int8 x,
  1282-wide blocks over 32 k-block partitions, bias seeded via s0, dequant
  via s1; a [128,4] mask matmul contracts the 32 partials per row.

DMA is split across all three rings (sync HWDGE / scalar HWDGE / gpsimd
SWDGE) with the TTR weights early so no engine starves.
"""

import numpy as np
import ml_dtypes

K = 41024            # features per side
NB = 321             # 128-wide k-blocks (block 320 is the 64-wide tail)
I = 8                # x-blocks per stationary load
NCH = 40             # full PE chunks (8 blocks each)
DGRP = 4             # PE chunks per cast group
NG8 = 4              # int8 cast groups (chunks 0..15)
RP = 40              # rows on the PE path
RT = 64 - RP         # rows on the DVE/TTR path
T = RT // 4          # TTR ops (4 rows each)
KB = 1282            # TTR block width (K = 32 * KB)
MOVP = I * RP        # moving columns per PE chunk
C8CH = NG8 * DGRP    # int8 chunks (16)
BFCH = NCH - C8CH    # bf16 chunks (24)
N_WARM = 14
N_CORES = 8
ROWS = 64

SW = 127.0 * np.sqrt(np.float64(K))   # W ~ U[-1/sqrt(K), 1/sqrt(K)]
SX = 127.0                            # x ~ U[0, 1)

# combined int8 const tensor (columns): xq | wq (T blocks) | wtail
XQ_OFF = 0
WQ_OFF = KB
WTL_OFF = KB + T * KB
C8_COLS = WTL_OFF + RP

_compiled = None


def _build_nc():
    import concourse.bacc as bacc
    import concourse.mybir as mybir
    import concourse.tile as tile
    from concourse.dve_ops import TENSOR_TENSOR_REDUCE

    F32 = mybir.dt.float32
    BF16 = mybir.dt.bfloat16
    I8 = mybir.dt.int8
    ADD = mybir.AluOpType.add
    MULT = mybir.AluOpType.mult

    nc = bacc.Bacc("TRN2", target_bir_lowering=False, debug=False)

    # int8 W for chunks 0..15 / bf16 W for chunks 16..39
    w8_d = nc.dram_tensor("w8", [128, C8CH * MOVP], I8, kind="ExternalInput")
    wbf_d = nc.dram_tensor("wbf", [128, BFCH * MOVP], BF16, kind="ExternalInput")
    c8_d = nc.dram_tensor("c8", [128, C8_COLS], I8, kind="ExternalInput")
    xs_d = nc.dram_tensor("xs", [128, NB], BF16, kind="ExternalInput")
    aux_d = nc.dram_tensor("aux", [128, 4 + T], F32, kind="ExternalInput")
    b_d = nc.dram_tensor("b", [1, RP], F32, kind="ExternalInput")
    sel_d = nc.dram_tensor("sel", [8, 8], BF16, kind="ExternalInput")
    ope_d = nc.dram_tensor("ope", [1, RP], F32, kind="ExternalOutput")
    otr_d = nc.dram_tensor("otr", [4, T], F32, kind="ExternalOutput")

    inv_sw = float(1.0 / SW)
    s1_ttr = float(1.0 / (SW * SX))
    GB = DGRP * MOVP              # columns per cast group

    with tile.TileContext(nc) as tc:
        with (
            tc.tile_pool(name="const", bufs=1) as constp,
            tc.tile_pool(name="w", bufs=1) as wp,
            tc.tile_pool(name="scratch", bufs=1) as sp,
            tc.tile_pool(name="ps", bufs=1, space="PSUM") as psp,
        ):
            # warm-up fodder: keep the PE p-state ramp alive from t=0
            # [128, 512] warm-ups pipeline at ~stream-rate, keeping the PE
            # continuously busy through the DMA-straggler window so the
            # p-state ramp reaches full clock before the real stream
            warm = constp.tile([128, 512], BF16, tag="warm")
            nc.gpsimd.memset(warm[:], 0.0)
            ps_w = psp.tile([1, 512], F32, tag="ps_w")
            for _ in range(N_WARM):
                nc.tensor.matmul(
                    ps_w[:], lhsT=warm[:, 0:1], rhs=warm[:], start=True, stop=True
                )

            c8 = constp.tile([128, C8_COLS], I8, tag="c8")
            xq = c8[:, XQ_OFF:WQ_OFF]
            xs = constp.tile([128, NB], BF16, tag="xs")
            aux = constp.tile([128, 4 + T], F32, tag="aux")
            sel = constp.tile([8, 8], BF16, tag="sel")
            bias = constp.tile([1, RP], F32, tag="bias")
            mask = aux[:, 0:4]
            seed = aux[:, 4 : 4 + T]

            ps = psp.tile([I, MOVP], F32, tag="ps")
            ps_t = psp.tile([1, RP], F32, tag="ps_t")
            ps_r = psp.tile([1, RP], F32, tag="ps_r")
            ps_m = psp.tile([4, T], F32, tag="ps_m")

            acc = constp.tile([128, T], F32, tag="acc")
            prod = sp.tile([128, KB], F32, tag="prod")
            w_tl = sp.tile([128, RP], BF16, tag="wtail")

            w8a = wp.tile([128, 2 * GB], I8, tag="w8a")      # groups 0-1
            w8b = wp.tile([128, 2 * GB], I8, tag="w8b")      # groups 2-3
            wbc = wp.tile([128, C8CH * MOVP], BF16, tag="wbc")  # cast target
            wbf1 = wp.tile([128, MOVP], BF16, tag="wbf1")
            wbf2 = wp.tile([128, 3 * MOVP], BF16, tag="wbf2")
            wbf3 = wp.tile([128, 8 * MOVP], BF16, tag="wbf3")
            wbf4 = wp.tile([128, 4 * MOVP], BF16, tag="wbf4")
            wbf5 = wp.tile([128, 4 * MOVP], BF16, tag="wbf5")
            wbf6 = wp.tile([128, 4 * MOVP], BF16, tag="wbf6")

            # ---- DMA schedule (3 rings in parallel).  The PE consumes
            # chunks ~2.5x faster than one ring delivers, so the bf16 W
            # stream is spread over all three rings: 16 chunks on sync,
            # 4 on gpsimd SWDGE (mid-stream), 4 on scalar (it goes idle
            # after the int8 groups).
            nc.sync.dma_start(xs[:], xs_d[:])
            nc.sync.dma_start(wbf1[:], wbf_d[:, :MOVP])
            nc.sync.dma_start(wbf2[:], wbf_d[:, MOVP : 4 * MOVP])
            nc.sync.dma_start(wbf3[:], wbf_d[:, 4 * MOVP : 12 * MOVP])
            nc.sync.dma_start(wbf4[:], wbf_d[:, 12 * MOVP : 16 * MOVP])
            # scalar HWDGE: xq + early wq first (feeds the DVE), rest of wq
            # + tail, the int8 W groups, then its share of late bf16 chunks
            nc.scalar.dma_start(c8[:, : 2 * KB], c8_d[:, : 2 * KB])
            nc.scalar.dma_start(w8a[:], w8_d[:, : 2 * GB])
            nc.scalar.dma_start(
                c8[:, 2 * KB : 4 * KB], c8_d[:, 2 * KB : 4 * KB]
            )
            nc.scalar.dma_start(
                c8[:, WQ_OFF + 3 * KB :], c8_d[:, WQ_OFF + 3 * KB :]
            )
            nc.scalar.dma_start(w8b[:], w8_d[:, 2 * GB :])
            nc.scalar.dma_start(wbf6[:], wbf_d[:, 20 * MOVP :])
            # gpsimd SWDGE: tiny constants + 4 mid-stream bf16 chunks
            nc.gpsimd.dma_start(aux[:], aux_d[:])
            nc.gpsimd.dma_start(wbf5[:], wbf_d[:, 16 * MOVP : 20 * MOVP])
            nc.gpsimd.dma_start(sel[:], sel_d[:])
            nc.gpsimd.dma_start(bias[:], b_d[:])

            # ---- ACT queue: the 4 group casts run uninterrupted; the tail
            # cast needs the late c8b DMA so it goes last
            for g in range(NG8):
                src = w8a if g < 2 else w8b
                off = (g % 2) * GB
                nc.scalar.copy(
                    wbc[:, g * GB : (g + 1) * GB], src[:, off : off + GB]
                )

            nc.scalar.copy(w_tl[:], c8[:, WTL_OFF:])

            # ---- DVE queue: TTRs in data-arrival order
            def ttr(t):
                nc.vector._custom_dve(
                    TENSOR_TENSOR_REDUCE,
                    out=prod[:],
                    in0=c8[:, WQ_OFF + t * KB : WQ_OFF + (t + 1) * KB],
                    in1=xq,
                    s0=seed[:, t : t + 1],
                    s1=s1_ttr,
                    accum_out=acc[:, t : t + 1],
                )

            for t in range(T):
                ttr(t)

            # ---- PE stream: bf16 chunks (16..39) first, then casted int8
            def mm(c, rhs, first, last):
                nc.tensor.matmul(
                    ps[:],
                    lhsT=xs[:, c * I : (c + 1) * I],
                    rhs=rhs,
                    start=first,
                    stop=last,
                )

            bf_tiles = [(wbf1, 1), (wbf2, 3), (wbf3, 8), (wbf4, 12)]
            c = C8CH
            for tile_, nch in bf_tiles:
                for k in range(nch):
                    mm(c, tile_[:, k * MOVP : (k + 1) * MOVP], c == C8CH, False)
                    c += 1
            for c in range(C8CH):
                mm(c, wbc[:, c * MOVP : (c + 1) * MOVP], False, c == C8CH - 1)

            nc.tensor.matmul(
                ps_t[:], lhsT=xs[:, NB - 1 : NB], rhs=w_tl[:], start=True, stop=True
            )
            nc.tensor.matmul(ps_m[:], lhsT=mask, rhs=acc[:], start=True, stop=True)

            otr_sb = sp.tile([4, T], F32, tag="otr")
            nc.vector.tensor_scalar_max(otr_sb[:], ps_m[:], 0.0)
            nc.gpsimd.dma_start(otr_d[:], otr_sb[:])

            # ---- PE-path extraction + fused scale/bias/relu
            sb8 = sp.tile([I, MOVP], BF16, tag="sb8")
            nc.vector.tensor_copy(sb8[:], ps[:])
            for j in range(I):
                nc.tensor.matmul(
                    ps_r[:],
                    lhsT=sel[:, j : j + 1],
                    rhs=sb8[:, j * RP : (j + 1) * RP],
                    start=(j == 0),
                    stop=(j == I - 1),
                )

            tb = sp.tile([1, RP], F32, tag="tb")
            nc.vector.scalar_tensor_tensor(
                tb[:], ps_t[:], inv_sw, bias[:], op0=MULT, op1=ADD
            )
            v = sp.tile([1, RP], F32, tag="v")
            nc.vector.scalar_tensor_tensor(
                v[:], ps_r[:], inv_sw, tb[:], op0=MULT, op1=ADD
            )
            ope_sb = sp.tile([1, RP], F32, tag="ope")
            nc.vector.tensor_scalar_max(ope_sb[:], v[:], 0.0)
            nc.sync.dma_start(ope_d[:], ope_sb[:])

    nc.compile()
    return nc


def _get_nc():
    global _compiled
    if _compiled is None:
        _compiled = _build_nc()
    return _compiled


def make_in_maps(input, W_my, b_my, W_opp, b_opp):
    """Host-side sharding + quantization: per-core input dicts."""
    x = np.ascontiguousarray(input, dtype=np.float32)
    Wcat = np.concatenate(
        [np.asarray(W_my, np.float32), np.asarray(W_opp, np.float32)], axis=0
    )
    bcat = np.concatenate(
        [np.asarray(b_my, np.float32), np.asarray(b_opp, np.float32)]
    )

    qW = np.clip(np.round(Wcat * SW), -127, 127).astype(np.int8)
    qx = np.clip(np.round(x * SX), -127, 127).astype(np.int8)
    sel = np.eye(8, dtype=ml_dtypes.bfloat16)
    mask = (np.arange(128)[:, None] // 32 == np.arange(4)[None, :]).astype(
        np.float32
    )

    in_maps = []
    for c in range(N_CORES):
        rows = slice(c * ROWS, (c + 1) * ROWS)
        qWsh = qW[rows]                      # [64, K] int8
        bsh = bcat[rows]
        xs_side = x[: K] if c < 4 else x[K:]
        qx_side = qx[: K] if c < 4 else qx[K:]

        # PE path, chunk-major: wall[p, c*MOVP + j*RP + r] = q[r, (c*8+j)*128+p]
        wall = (
            qWsh[:RP, : NCH * I * 128]
            .reshape(RP, NCH, I, 128)
            .transpose(3, 1, 2, 0)
            .reshape(128, NCH * MOVP)
        )
        w8 = np.ascontiguousarray(wall[:, : C8CH * MOVP])           # int8
        wbf = wall[:, C8CH * MOVP :].astype(ml_dtypes.bfloat16)     # bf16(q)

        xp = np.zeros(NB * 128, np.float32)
        xp[:K] = xs_side
        xs = np.ascontiguousarray(xp.reshape(NB, 128).T).astype(ml_dtypes.bfloat16)

        # combined int8 consts: xq | wq | wtail (tail = q, zero-padded)
        c8 = np.zeros((128, C8_COLS), np.int8)
        c8[:, XQ_OFF:WQ_OFF] = np.tile(qx_side.reshape(32, KB), (4, 1))
        c8[:, WQ_OFF:WTL_OFF] = (
            qWsh[RP:].reshape(T, 4, 32, KB).transpose(1, 2, 0, 3).reshape(128, T * KB)
        )
        c8[:64, WTL_OFF:] = qWsh[:RP, NCH * I * 128 :].T

        aux = np.zeros((128, 4 + T), np.float32)
        aux[:, 0:4] = mask
        seed = np.zeros((128, T), np.float32)
        seed[np.arange(4) * 32, :] = bsh[RP:].reshape(T, 4).T
        aux[:, 4:] = seed

        b = np.ascontiguousarray(bsh[:RP].reshape(1, RP))
        in_maps.append(
            {"w8": w8, "wbf": wbf, "c8": c8, "xs": xs, "aux": aux, "b": b,
             "sel": sel}
        )
    return in_maps


def gather_output(results):
    """per-core: 'ope' [1, RP] rows 0..RP-1, 'otr' [4, T] row RP + t*4 + rr."""
    outs = []
    for c in range(N_CORES):
        pe = np.asarray(results[c]["ope"], np.float32).ravel()
        tr = np.asarray(results[c]["otr"], np.float32).T.ravel()
        outs.append(np.concatenate([pe, tr]))
    return np.concatenate(outs)


def run_on_hw(in_maps, trace=False, **kwargs):
    from concourse.bass_utils import run_bass_kernel_spmd

    nc = _get_nc()
    return run_bass_kernel_spmd(
        nc, in_maps, core_ids=list(range(N_CORES)), trace=trace, **kwargs
    )


def kernel(input, W_my, b_my, W_opp, b_opp):
    in_maps = make_in_maps(input, W_my, b_my, W_opp, b_opp)
    res = run_on_hw(in_maps)
    return gather_output(res.results)


# revision 43
# speedup vs baseline: 1.0011x; 1.0011x over previous
"""HalfKP input layer (dual GEMV + bias + relu) on 8 Trainium2 NeuronCores.

out[512] = concat(relu(W_my @ x[:41024] + b_my), relu(W_opp @ x[41024:] + b_opp))

Sharding: 512 output rows split 64 rows/core (cores 0-3: W_my, 4-7: W_opp).
W is quantized to q = round(W * 127 * sqrt(K)) (rel err ~5e-3, gate is 2e-2)
and shipped partly as int8, partly as bf16(q) — same values, one uniform
1/sw dequant at the end.

Per core the 64 rows split across two pipelines that run in parallel
(measured: ACT cast ~0.8-1.0 G elem/s/partition, DVE cast 1.6, DVE TTR 0.83,
PE ~1.13 GHz under the activity throttle; HWDGE DMA ~250 GB/s/ring):

* PE path (rows 0..RP-1): x-stationary GEMV.  K cut into 321 blocks of 128
  (tail zero-padded).  Per chunk of 8 blocks the PE loads 8 x-blocks (bf16)
  stationary and streams the chunk's W as the moving operand [128, 8*RP],
  accumulating psum[i, n] over 40 chunks.  Chunks 16..39 ship as bf16
  directly (no cast, DMA-paced); chunks 0..15 ship int8 and are cast to
  bf16 by the Scalar engine in 4 group-casts, overlapping the bf16 DMA.
  The bf16 chunks are processed FIRST so the PE starts as soon as the
  first bf16 group lands.  Diagonal extraction: one aligned psum->SBUF
  copy + 8 selector matmuls (lhsT = e_j), then a fused scale+bias DVE op
  and relu.

* DVE path (rows RP..63): TENSOR_TENSOR_REDUCE directly on int8 W x int8 x,
  1282-wide blocks over 32 k-block partitions, bias seeded via s0, dequant
  via s1; a [128,4] mask matmul contracts the 32 partials per row.

DMA is split across all three rings (sync HWDGE / scalar HWDGE / gpsimd
SWDGE) with the TTR weights early so no engine starves.
"""

import numpy as np
import ml_dtypes

K = 41024            # features per side
NB = 321             # 128-wide k-blocks (block 320 is the 64-wide tail)
I = 8                # x-blocks per stationary load
NCH = 40             # full PE chunks (8 blocks each)
DGRP = 4             # PE chunks per cast group
NG8 = 4              # int8 cast groups (chunks 0..15)
RP = 40              # rows on the PE path
RT = 64 - RP         # rows on the DVE/TTR path
T = RT // 4          # TTR ops (4 rows each)
KB = 1282            # TTR block width (K = 32 * KB)
MOVP = I * RP        # moving columns per PE chunk
C8CH = NG8 * DGRP    # int8 chunks (16)
BFCH = NCH - C8CH    # bf16 chunks (24)
N_WARM = 14
N_CORES = 8
ROWS = 64

SW = 127.0 * np.sqrt(np.float64(K))   # W ~ U[-1/sqrt(K), 1/sqrt(K)]
SX = 127.0                            # x ~ U[0, 1)

# combined int8 const tensor (columns): xq | wq (T blocks) | wtail
XQ_OFF = 0
WQ_OFF = KB
WTL_OFF = KB + T * KB
C8_COLS = WTL_OFF + RP

_compiled = None


def _build_nc():
    import concourse.bacc as bacc
    import concourse.mybir as mybir
    import concourse.tile as tile
    from concourse.dve_ops import TENSOR_TENSOR_REDUCE

    F32 = mybir.dt.float32
    BF16 = mybir.dt.bfloat16
    I8 = mybir.dt.int8
    ADD = mybir.AluOpType.add
    MULT = mybir.AluOpType.mult

    nc = bacc.Bacc("TRN2", target_bir_lowering=False, debug=False)

    # int8 W for chunks 0..15 / bf16 W for chunks 16..39
    w8_d = nc.dram_tensor("w8", [128, C8CH * MOVP], I8, kind="ExternalInput")
    wbf_d = nc.dram_tensor("wbf", [128, BFCH * MOVP], BF16, kind="ExternalInput")
    c8_d = nc.dram_tensor("c8", [128, C8_COLS], I8, kind="ExternalInput")
    xs_d = nc.dram_tensor("xs", [128, NB], BF16, kind="ExternalInput")
    aux_d = nc.dram_tensor("aux", [128, 4 + T], F32, kind="ExternalInput")
    b_d = nc.dram_tensor("b", [1, RP], F32, kind="ExternalInput")
    sel_d = nc.dram_tensor("sel", [8, 8], BF16, kind="ExternalInput")
    ope_d = nc.dram_tensor("ope", [1, RP], F32, kind="ExternalOutput")
    otr_d = nc.dram_tensor("otr", [4, T], F32, kind="ExternalOutput")

    inv_sw = float(1.0 / SW)
    s1_ttr = float(1.0 / (SW * SX))
    GB = DGRP * MOVP              # columns per cast group

    with tile.TileContext(nc) as tc:
        with (
            tc.tile_pool(name="const", bufs=1) as constp,
            tc.tile_pool(name="w", bufs=1) as wp,
            tc.tile_pool(name="scratch", bufs=1) as sp,
            tc.tile_pool(name="ps", bufs=1, space="PSUM") as psp,
        ):
            # warm-up fodder: keep the PE p-state ramp alive from t=0
            # [128, 512] warm-ups pipeline at ~stream-rate, keeping the PE
            # continuously busy through the DMA-straggler window so the
            # p-state ramp reaches full clock before the real stream
            warm = constp.tile([128, 512], BF16, tag="warm")
            nc.gpsimd.memset(warm[:], 0.0)
            ps_w = psp.tile([1, 512], F32, tag="ps_w")
            for _ in range(N_WARM):
                nc.tensor.matmul(
                    ps_w[:], lhsT=warm[:, 0:1], rhs=warm[:], start=True, stop=True
                )

            c8 = constp.tile([128, C8_COLS], I8, tag="c8")
            xq = c8[:, XQ_OFF:WQ_OFF]
            xs = constp.tile([128, NB], BF16, tag="xs")
            aux = constp.tile([128, 4 + T], F32, tag="aux")
            sel = constp.tile([8, 8], BF16, tag="sel")
            bias = constp.tile([1, RP], F32, tag="bias")
            mask = aux[:, 0:4]
            seed = aux[:, 4 : 4 + T]

            ps = psp.tile([I, MOVP], F32, tag="ps")
            ps_t = psp.tile([1, RP], F32, tag="ps_t")
            ps_r = psp.tile([1, RP], F32, tag="ps_r")
            ps_m = psp.tile([4, T], F32, tag="ps_m")

            acc = constp.tile([128, T], F32, tag="acc")
            prod = sp.tile([128, KB], F32, tag="prod")
            w_tl = sp.tile([128, RP], BF16, tag="wtail")

            w8a = wp.tile([128, 2 * GB], I8, tag="w8a")      # groups 0-1
            w8b = wp.tile([128, 2 * GB], I8, tag="w8b")      # groups 2-3
            wbc = wp.tile([128, C8CH * MOVP], BF16, tag="wbc")  # cast target
            wbf1 = wp.tile([128, MOVP], BF16, tag="wbf1")
            wbf2 = wp.tile([128, 3 * MOVP], BF16, tag="wbf2")
            wbf3 = wp.tile([128, 8 * MOVP], BF16, tag="wbf3")
            wbf4 = wp.tile([128, 4 * MOVP], BF16, tag="wbf4")
            wbf5 = wp.tile([128, 4 * MOVP], BF16, tag="wbf5")
            wbf6 = wp.tile([128, 4 * MOVP], BF16, tag="wbf6")

            # ---- DMA schedule (3 rings in parallel).  The PE consumes
            # chunks ~2.5x faster than one ring delivers, so the bf16 W
            # stream is spread over all three rings: 16 chunks on sync,
            # 4 on gpsimd SWDGE (mid-stream), 4 on scalar (it goes idle
            # after the int8 groups).
            nc.sync.dma_start(xs[:], xs_d[:])
            nc.sync.dma_start(wbf1[:], wbf_d[:, :MOVP])
            nc.sync.dma_start(wbf2[:], wbf_d[:, MOVP : 4 * MOVP])
            nc.sync.dma_start(wbf3[:], wbf_d[:, 4 * MOVP : 12 * MOVP])
            nc.sync.dma_start(wbf4[:], wbf_d[:, 12 * MOVP : 16 * MOVP])
            # scalar HWDGE: xq + early wq first (feeds the DVE), rest of wq
            # + tail, the int8 W groups, then its share of late bf16 chunks
            nc.scalar.dma_start(
                c8[:, : WQ_OFF + 3 * KB], c8_d[:, : WQ_OFF + 3 * KB]
            )
            nc.scalar.dma_start(w8a[:], w8_d[:, : 2 * GB])
            nc.scalar.dma_start(
                c8[:, WQ_OFF + 3 * KB :], c8_d[:, WQ_OFF + 3 * KB :]
            )
            nc.scalar.dma_start(w8b[:], w8_d[:, 2 * GB :])
            nc.scalar.dma_start(wbf6[:], wbf_d[:, 20 * MOVP :])
            # gpsimd SWDGE: tiny constants + 4 mid-stream bf16 chunks
            nc.gpsimd.dma_start(aux[:], aux_d[:])
            nc.gpsimd.dma_start(wbf5[:], wbf_d[:, 16 * MOVP : 20 * MOVP])
            nc.gpsimd.dma_start(sel[:], sel_d[:])
            nc.gpsimd.dma_start(bias[:], b_d[:])

            # ---- ACT queue: the 4 group casts run uninterrupted; the tail
            # cast needs the late c8b DMA so it goes last
            for g in range(NG8):
                src = w8a if g < 2 else w8b
                off = (g % 2) * GB
                nc.scalar.copy(
                    wbc[:, g * GB : (g + 1) * GB], src[:, off : off + GB]
                )

            nc.scalar.copy(w_tl[:], c8[:, WTL_OFF:])

            # ---- DVE queue: TTRs in data-arrival order
            def ttr(t):
                nc.vector._custom_dve(
                    TENSOR_TENSOR_REDUCE,
                    out=prod[:],
                    in0=c8[:, WQ_OFF + t * KB : WQ_OFF + (t + 1) * KB],
                    in1=xq,
                    s0=seed[:, t : t + 1],
                    s1=s1_ttr,
                    accum_out=acc[:, t : t + 1],
                )

            for t in range(T):
                ttr(t)

            # ---- PE stream: bf16 chunks (16..39) first, then casted int8
            def mm(c, rhs, first, last):
                nc.tensor.matmul(
                    ps[:],
                    lhsT=xs[:, c * I : (c + 1) * I],
                    rhs=rhs,
                    start=first,
                    stop=last,
                )

            bf_tiles = [(wbf1, 1), (wbf2, 3), (wbf3, 8), (wbf4, 12)]
            c = C8CH
            for tile_, nch in bf_tiles:
                for k in range(nch):
                    mm(c, tile_[:, k * MOVP : (k + 1) * MOVP], c == C8CH, False)
                    c += 1
            for c in range(C8CH):
                mm(c, wbc[:, c * MOVP : (c + 1) * MOVP], False, c == C8CH - 1)

            nc.tensor.matmul(
                ps_t[:], lhsT=xs[:, NB - 1 : NB], rhs=w_tl[:], start=True, stop=True
            )
            nc.tensor.matmul(ps_m[:], lhsT=mask, rhs=acc[:], start=True, stop=True)

            otr_sb = sp.tile([4, T], F32, tag="otr")
            nc.vector.tensor_scalar_max(otr_sb[:], ps_m[:], 0.0)
            nc.gpsimd.dma_start(otr_d[:], otr_sb[:])

            # ---- PE-path extraction + fused scale/bias/relu
            sb8 = sp.tile([I, MOVP], BF16, tag="sb8")
            nc.vector.tensor_copy(sb8[:], ps[:])
            for j in range(I):
                nc.tensor.matmul(
                    ps_r[:],
                    lhsT=sel[:, j : j + 1],
                    rhs=sb8[:, j * RP : (j + 1) * RP],
                    start=(j == 0),
                    stop=(j == I - 1),
                )

            tb = sp.tile([1, RP], F32, tag="tb")
            nc.vector.scalar_tensor_tensor(
                tb[:], ps_t[:], inv_sw, bias[:], op0=MULT, op1=ADD
            )
            v = sp.tile([1, RP], F32, tag="v")
            nc.vector.scalar_tensor_tensor(
                v[:], ps_r[:], inv_sw, tb[:], op0=MULT, op1=ADD
            )
            ope_sb = sp.tile([1, RP], F32, tag="ope")
            nc.vector.tensor_scalar_max(ope_sb[:], v[:], 0.0)
            nc.sync.dma_start(ope_d[:], ope_sb[:])

    nc.compile()
    return nc


def _get_nc():
    global _compiled
    if _compiled is None:
        _compiled = _build_nc()
    return _compiled


def make_in_maps(input, W_my, b_my, W_opp, b_opp):
    """Host-side sharding + quantization: per-core input dicts."""
    x = np.ascontiguousarray(input, dtype=np.float32)
    Wcat = np.concatenate(
        [np.asarray(W_my, np.float32), np.asarray(W_opp, np.float32)], axis=0
    )
    bcat = np.concatenate(
        [np.asarray(b_my, np.float32), np.asarray(b_opp, np.float32)]
    )

    qW = np.clip(np.round(Wcat * SW), -127, 127).astype(np.int8)
    qx = np.clip(np.round(x * SX), -127, 127).astype(np.int8)
    sel = np.eye(8, dtype=ml_dtypes.bfloat16)
    mask = (np.arange(128)[:, None] // 32 == np.arange(4)[None, :]).astype(
        np.float32
    )

    in_maps = []
    for c in range(N_CORES):
        rows = slice(c * ROWS, (c + 1) * ROWS)
        qWsh = qW[rows]                      # [64, K] int8
        bsh = bcat[rows]
        xs_side = x[: K] if c < 4 else x[K:]
        qx_side = qx[: K] if c < 4 else qx[K:]

        # PE path, chunk-major: wall[p, c*MOVP + j*RP + r] = q[r, (c*8+j)*128+p]
        wall = (
            qWsh[:RP, : NCH * I * 128]
            .reshape(RP, NCH, I, 128)
            .transpose(3, 1, 2, 0)
            .reshape(128, NCH * MOVP)
        )
        w8 = np.ascontiguousarray(wall[:, : C8CH * MOVP])           # int8
        wbf = wall[:, C8CH * MOVP :].astype(ml_dtypes.bfloat16)     # bf16(q)

        xp = np.zeros(NB * 128, np.float32)
        xp[:K] = xs_side
        xs = np.ascontiguousarray(xp.reshape(NB, 128).T).astype(ml_dtypes.bfloat16)

        # combined int8 consts: xq | wq | wtail (tail = q, zero-padded)
        c8 = np.zeros((128, C8_COLS), np.int8)
        c8[:, XQ_OFF:WQ_OFF] = np.tile(qx_side.reshape(32, KB), (4, 1))
        c8[:, WQ_OFF:WTL_OFF] = (
            qWsh[RP:].reshape(T, 4, 32, KB).transpose(1, 2, 0, 3).reshape(128, T * KB)
        )
        c8[:64, WTL_OFF:] = qWsh[:RP, NCH * I * 128 :].T

        aux = np.zeros((128, 4 + T), np.float32)
        aux[:, 0:4] = mask
        seed = np.zeros((128, T), np.float32)
        seed[np.arange(4) * 32, :] = bsh[RP:].reshape(T, 4).T
        aux[:, 4:] = seed

        b = np.ascontiguousarray(bsh[:RP].reshape(1, RP))
        in_maps.append(
            {"w8": w8, "wbf": wbf, "c8": c8, "xs": xs, "aux": aux, "b": b,
             "sel": sel}
        )
    return in_maps


def gather_output(results):
    """per-core: 'ope' [1, RP] rows 0..RP-1, 'otr' [4, T] row RP + t*4 + rr."""
    outs = []
    for c in range(N_CORES):
        pe = np.asarray(results[c]["ope"], np.float32).ravel()
        tr = np.asarray(results[c]["otr"], np.float32).T.ravel()
        outs.append(np.concatenate([pe, tr]))
    return np.concatenate(outs)


def run_on_hw(in_maps, trace=False, **kwargs):
    from concourse.bass_utils import run_bass_kernel_spmd

    nc = _get_nc()
    return run_bass_kernel_spmd(
        nc, in_maps, core_ids=list(range(N_CORES)), trace=trace, **kwargs
    )


def kernel(input, W_my, b_my, W_opp, b_opp):
    in_maps = make_in_maps(input, W_my, b_my, W_opp, b_opp)
    res = run_on_hw(in_maps)
    return gather_output(res.results)


# revision 45
# speedup vs baseline: 1.0060x; 1.0049x over previous
"""HalfKP input layer (dual GEMV + bias + relu) on 8 Trainium2 NeuronCores.

out[512] = concat(relu(W_my @ x[:41024] + b_my), relu(W_opp @ x[41024:] + b_opp))

Sharding: 512 output rows split 64 rows/core (cores 0-3: W_my, 4-7: W_opp).
W is quantized to q = round(W * 127 * sqrt(K)) (rel err ~5e-3, gate is 2e-2)
and shipped partly as int8, partly as bf16(q) — same values, one uniform
1/sw dequant at the end.

Per core the 64 rows split across two pipelines that run in parallel
(measured: ACT cast ~0.8-1.0 G elem/s/partition, DVE cast 1.6, DVE TTR 0.83,
PE ~1.13 GHz under the activity throttle; HWDGE DMA ~250 GB/s/ring):

* PE path (rows 0..RP-1): x-stationary GEMV.  K cut into 321 blocks of 128
  (tail zero-padded).  Per chunk of 8 blocks the PE loads 8 x-blocks (bf16)
  stationary and streams the chunk's W as the moving operand [128, 8*RP],
  accumulating psum[i, n] over 40 chunks.  Chunks 16..39 ship as bf16
  directly (no cast, DMA-paced); chunks 0..15 ship int8 and are cast to
  bf16 by the Scalar engine in 4 group-casts, overlapping the bf16 DMA.
  The bf16 chunks are processed FIRST so the PE starts as soon as the
  first bf16 group lands.  Diagonal extraction: one aligned psum->SBUF
  copy + 8 selector matmuls (lhsT = e_j), then a fused scale+bias DVE op
  and relu.

* DVE path (rows RP..63): TENSOR_TENSOR_REDUCE directly on int8 W x int8 x,
  1282-wide blocks over 32 k-block partitions, bias seeded via s0, dequant
  via s1; a [128,4] mask matmul contracts the 32 partials per row.

DMA is split across all three rings (sync HWDGE / scalar HWDGE / gpsimd
SWDGE) with the TTR weights early so no engine starves.
"""

import numpy as np
import ml_dtypes

K = 41024            # features per side
NB = 321             # 128-wide k-blocks (block 320 is the 64-wide tail)
I = 8                # x-blocks per stationary load
NCH = 40             # full PE chunks (8 blocks each)
DGRP = 4             # PE chunks per cast group
NG8 = 4              # int8 cast groups (chunks 0..15)
RP = 40              # rows on the PE path
RT = 64 - RP         # rows on the DVE/TTR path
T = RT // 4          # TTR ops (4 rows each)
KB = 1282            # TTR block width (K = 32 * KB)
MOVP = I * RP        # moving columns per PE chunk
C8CH = NG8 * DGRP    # int8 chunks (16)
BFCH = NCH - C8CH    # bf16 chunks (24)
N_WARM = 14
N_CORES = 8
ROWS = 64

SW = 127.0 * np.sqrt(np.float64(K))   # W ~ U[-1/sqrt(K), 1/sqrt(K)]
SX = 127.0                            # x ~ U[0, 1)

# combined int8 const tensor (columns): xq | wq (T blocks) | wtail
XQ_OFF = 0
WQ_OFF = KB
WTL_OFF = KB + T * KB
C8_COLS = WTL_OFF + RP

_compiled = None


def _build_nc():
    import concourse.bacc as bacc
    import concourse.mybir as mybir
    import concourse.tile as tile
    from concourse.dve_ops import TENSOR_TENSOR_REDUCE

    F32 = mybir.dt.float32
    BF16 = mybir.dt.bfloat16
    I8 = mybir.dt.int8
    ADD = mybir.AluOpType.add
    MULT = mybir.AluOpType.mult

    nc = bacc.Bacc("TRN2", target_bir_lowering=False, debug=False)

    # int8 W for chunks 0..15 / bf16 W for chunks 16..39
    w8_d = nc.dram_tensor("w8", [128, C8CH * MOVP], I8, kind="ExternalInput")
    wbf_d = nc.dram_tensor("wbf", [128, BFCH * MOVP], BF16, kind="ExternalInput")
    c8_d = nc.dram_tensor("c8", [128, C8_COLS], I8, kind="ExternalInput")
    xs_d = nc.dram_tensor("xs", [128, NB], BF16, kind="ExternalInput")
    aux_d = nc.dram_tensor("aux", [128, 4 + T], F32, kind="ExternalInput")
    b_d = nc.dram_tensor("b", [1, RP], F32, kind="ExternalInput")
    sel_d = nc.dram_tensor("sel", [8, 8], BF16, kind="ExternalInput")
    ope_d = nc.dram_tensor("ope", [1, RP], F32, kind="ExternalOutput")
    otr_d = nc.dram_tensor("otr", [4, T], F32, kind="ExternalOutput")

    inv_sw = float(1.0 / SW)
    s1_ttr = float(1.0 / (SW * SX))
    GB = DGRP * MOVP              # columns per cast group

    with tile.TileContext(nc) as tc:
        with (
            tc.tile_pool(name="const", bufs=1) as constp,
            tc.tile_pool(name="w", bufs=1) as wp,
            tc.tile_pool(name="scratch", bufs=1) as sp,
            tc.tile_pool(name="ps", bufs=1, space="PSUM") as psp,
        ):
            # warm-up fodder: keep the PE p-state ramp alive from t=0
            # [128, 512] warm-ups pipeline at ~stream-rate, keeping the PE
            # continuously busy through the DMA-straggler window so the
            # p-state ramp reaches full clock before the real stream
            warm = constp.tile([128, 512], BF16, tag="warm")
            nc.gpsimd.memset(warm[:], 0.0)
            ps_w = psp.tile([1, 512], F32, tag="ps_w")
            for _ in range(N_WARM):
                nc.tensor.matmul(
                    ps_w[:], lhsT=warm[:, 0:1], rhs=warm[:], start=True, stop=True
                )

            c8 = constp.tile([128, C8_COLS], I8, tag="c8")
            xq = c8[:, XQ_OFF:WQ_OFF]
            xs = constp.tile([128, NB], BF16, tag="xs")
            aux = constp.tile([128, 4 + T], F32, tag="aux")
            sel = constp.tile([8, 8], BF16, tag="sel")
            bias = constp.tile([1, RP], F32, tag="bias")
            mask = aux[:, 0:4]
            seed = aux[:, 4 : 4 + T]

            ps = psp.tile([I, MOVP], F32, tag="ps")
            ps_t = psp.tile([1, RP], F32, tag="ps_t")
            ps_r = psp.tile([1, RP], F32, tag="ps_r")
            ps_m = psp.tile([4, T], F32, tag="ps_m")

            acc = constp.tile([128, T], F32, tag="acc")
            prod = sp.tile([128, KB], F32, tag="prod")
            w_tl = sp.tile([128, RP], BF16, tag="wtail")

            w8a = wp.tile([128, 2 * GB], I8, tag="w8a")      # groups 0-1
            w8b = wp.tile([128, 2 * GB], I8, tag="w8b")      # groups 2-3
            wbc = wp.tile([128, C8CH * MOVP], BF16, tag="wbc")  # cast target
            wbf1 = wp.tile([128, MOVP], BF16, tag="wbf1")
            wbf2 = wp.tile([128, 3 * MOVP], BF16, tag="wbf2")
            wbf3 = wp.tile([128, 8 * MOVP], BF16, tag="wbf3")
            wbf4 = wp.tile([128, 4 * MOVP], BF16, tag="wbf4")
            wbf5 = wp.tile([128, 4 * MOVP], BF16, tag="wbf5")
            wbf6 = wp.tile([128, 4 * MOVP], BF16, tag="wbf6")

            # ---- DMA schedule (3 rings in parallel).  The PE consumes
            # chunks ~2.5x faster than one ring delivers, so the bf16 W
            # stream is spread over all three rings: 16 chunks on sync,
            # 4 on gpsimd SWDGE (mid-stream), 4 on scalar (it goes idle
            # after the int8 groups).
            nc.sync.dma_start(xs[:], xs_d[:])
            nc.sync.dma_start(wbf1[:], wbf_d[:, :MOVP])
            nc.sync.dma_start(wbf2[:], wbf_d[:, MOVP : 4 * MOVP])
            nc.sync.dma_start(wbf3[:], wbf_d[:, 4 * MOVP : 12 * MOVP])
            nc.sync.dma_start(wbf4[:], wbf_d[:, 12 * MOVP : 16 * MOVP])
            # scalar HWDGE: xq + early wq first (feeds the DVE), rest of wq
            # + tail, the int8 W groups, then its share of late bf16 chunks
            nc.scalar.dma_start(
                c8[:, : WQ_OFF + 3 * KB], c8_d[:, : WQ_OFF + 3 * KB]
            )
            nc.scalar.dma_start(w8a[:], w8_d[:, : 2 * GB])
            nc.scalar.dma_start(
                c8[:, WQ_OFF + 3 * KB :], c8_d[:, WQ_OFF + 3 * KB :]
            )
            nc.scalar.dma_start(w8b[:], w8_d[:, 2 * GB :])
            nc.scalar.dma_start(wbf6[:], wbf_d[:, 20 * MOVP :])
            # gpsimd SWDGE: tiny constants + 4 mid-stream bf16 chunks
            nc.gpsimd.dma_start(aux[:], aux_d[:])
            nc.gpsimd.dma_start(wbf5[:], wbf_d[:, 16 * MOVP : 20 * MOVP])
            nc.gpsimd.dma_start(sel[:], sel_d[:])
            nc.gpsimd.dma_start(bias[:], b_d[:])

            # ---- ACT queue: the 4 group casts run uninterrupted; the tail
            # cast needs the late c8b DMA so it goes last
            for g in range(NG8):
                src = w8a if g < 2 else w8b
                off = (g % 2) * GB
                nc.scalar.copy(
                    wbc[:, g * GB : (g + 1) * GB], src[:, off : off + GB]
                )

            nc.scalar.copy(w_tl[:], c8[:, WTL_OFF:])

            # ---- DVE queue: TTRs in data-arrival order
            def ttr(t):
                nc.vector._custom_dve(
                    TENSOR_TENSOR_REDUCE,
                    out=prod[:],
                    in0=c8[:, WQ_OFF + t * KB : WQ_OFF + (t + 1) * KB],
                    in1=xq,
                    s0=seed[:, t : t + 1],
                    s1=s1_ttr,
                    accum_out=acc[:, t : t + 1],
                )

            for t in range(T):
                ttr(t)

            # ---- PE stream: bf16 chunks (16..39) first, then casted int8
            def mm(c, rhs, first, last):
                nc.tensor.matmul(
                    ps[:],
                    lhsT=xs[:, c * I : (c + 1) * I],
                    rhs=rhs,
                    start=first,
                    stop=last,
                )

            bf_tiles = [(wbf1, 1), (wbf2, 3), (wbf3, 8), (wbf4, 12)]
            c = C8CH
            for tile_, nch in bf_tiles:
                for k in range(nch):
                    mm(c, tile_[:, k * MOVP : (k + 1) * MOVP], c == C8CH, False)
                    c += 1
            for c in range(C8CH):
                mm(c, wbc[:, c * MOVP : (c + 1) * MOVP], False, c == C8CH - 1)

            nc.tensor.matmul(
                ps_t[:], lhsT=xs[:, NB - 1 : NB], rhs=w_tl[:], start=True, stop=True
            )
            nc.tensor.matmul(ps_m[:], lhsT=mask, rhs=acc[:], start=True, stop=True)

            otr_sb = sp.tile([4, T], F32, tag="otr")
            nc.vector.tensor_scalar_max(otr_sb[:], ps_m[:], 0.0)
            nc.gpsimd.dma_start(otr_d[:], otr_sb[:])

            # ---- PE-path extraction + fused scale/bias/relu
            sb8 = sp.tile([I, MOVP], BF16, tag="sb8")
            nc.vector.tensor_copy(sb8[:], ps[:])
            for j in range(I):
                nc.tensor.matmul(
                    ps_r[:],
                    lhsT=sel[:, j : j + 1],
                    rhs=sb8[:, j * RP : (j + 1) * RP],
                    start=(j == 0),
                    stop=(j == I - 1),
                )

            tb = sp.tile([1, RP], F32, tag="tb")
            nc.vector.scalar_tensor_tensor(
                tb[:], ps_t[:], inv_sw, bias[:], op0=MULT, op1=ADD
            )
            v = sp.tile([1, RP], F32, tag="v")
            nc.vector.scalar_tensor_tensor(
                v[:], ps_r[:], inv_sw, tb[:], op0=MULT, op1=ADD
            )
            ope_sb = sp.tile([1, RP], F32, tag="ope")
            nc.vector.tensor_scalar_max(ope_sb[:], v[:], 0.0)
            nc.sync.dma_start(ope_d[:], ope_sb[:])

    nc.compile()
    return nc


def _get_nc():
    global _compiled
    if _compiled is None:
        _compiled = _build_nc()
    return _compiled


def make_in_maps(input, W_my, b_my, W_opp, b_opp):
    """Host-side sharding + quantization: per-core input dicts."""
    x = np.ascontiguousarray(input, dtype=np.float32)
    Wcat = np.concatenate(
        [np.asarray(W_my, np.float32), np.asarray(W_opp, np.float32)], axis=0
    )
    bcat = np.concatenate(
        [np.asarray(b_my, np.float32), np.asarray(b_opp, np.float32)]
    )

    qW = np.clip(np.round(Wcat * SW), -127, 127).astype(np.int8)
    qx = np.clip(np.round(x * SX), -127, 127).astype(np.int8)
    sel = np.eye(8, dtype=ml_dtypes.bfloat16)
    mask = (np.arange(128)[:, None] // 32 == np.arange(4)[None, :]).astype(
        np.float32
    )

    in_maps = []
    for c in range(N_CORES):
        rows = slice(c * ROWS, (c + 1) * ROWS)
        qWsh = qW[rows]                      # [64, K] int8
        bsh = bcat[rows]
        xs_side = x[: K] if c < 4 else x[K:]
        qx_side = qx[: K] if c < 4 else qx[K:]

        # PE path, chunk-major: wall[p, c*MOVP + j*RP + r] = q[r, (c*8+j)*128+p]
        wall = (
            qWsh[:RP, : NCH * I * 128]
            .reshape(RP, NCH, I, 128)
            .transpose(3, 1, 2, 0)
            .reshape(128, NCH * MOVP)
        )
        w8 = np.ascontiguousarray(wall[:, : C8CH * MOVP])           # int8
        wbf = wall[:, C8CH * MOVP :].astype(ml_dtypes.bfloat16)     # bf16(q)

        xp = np.zeros(NB * 128, np.float32)
        xp[:K] = xs_side
        xs = np.ascontiguousarray(xp.reshape(NB, 128).T).astype(ml_dtypes.bfloat16)

        # combined int8 consts: xq | wq | wtail (tail = q, zero-padded)
        c8 = np.zeros((128, C8_COLS), np.int8)
        c8[:, XQ_OFF:WQ_OFF] = np.tile(qx_side.reshape(32, KB), (4, 1))
        c8[:, WQ_OFF:WTL_OFF] = (
            qWsh[RP:].reshape(T, 4, 32, KB).transpose(1, 2, 0, 3).reshape(128, T * KB)
        )
        c8[:64, WTL_OFF:] = qWsh[:RP, NCH * I * 128 :].T

        aux = np.zeros((128, 4 + T), np.float32)
        aux[:, 0:4] = mask
        seed = np.zeros((128, T), np.float32)
        seed[np.arange(4) * 32, :] = bsh[RP:].reshape(T, 4).T
        aux[:, 4:] = seed

        b = np.ascontiguousarray(bsh[:RP].reshape(1, RP))
        in_maps.append(
            {"w8": w8, "wbf": wbf, "c8": c8, "xs": xs, "aux": aux, "b": b,
             "sel": sel}
        )
    return in_maps


def gather_output(results):
    """per-core: 'ope' [1, RP] rows 0..RP-1, 'otr' [4, T] row RP + t*4 + rr."""
    outs = []
    for c in range(N_CORES):
        pe = np.asarray(results[c]["ope"], np.float32).ravel()
        tr = np.asarray(results[c]["otr"], np.float32).T.ravel()
        outs.append(np.concatenate([pe, tr]))
    return np.concatenate(outs)


def run_on_hw(in_maps, trace=False, **kwargs):
    from concourse.bass_utils import run_bass_kernel_spmd

    nc = _get_nc()
    return run_bass_kernel_spmd(
        nc, in_maps, core_ids=list(range(N_CORES)), trace=trace, **kwargs
    )


def kernel(input, W_my, b_my, W_opp, b_opp):
    in_maps = make_in_maps(input, W_my, b_my, W_opp, b_opp)
    res = run_on_hw(in_maps)
    return gather_output(res.results)
